# revision 11
# baseline (speedup 1.0000x reference)
"""Paged-attention decode (GQA, vLLM-style) on 8 TRN2 NeuronCores.

Sharding: kv-head-parallel — core c owns kv-head c (and its 4 query heads) for
ALL 16 sequences; no collectives.  Each core processes 16 slabs, one per
(sequence, head) unit, in descending context-length order.  Because a slab is
a single sequence, the graph's per-slab kv extent is exactly that sequence's
ctx-1 valid rows (the final 128-tile is partial) — invalid kv is never loaded
nor computed, which also makes any masking unnecessary.  The graph is compiled
per call (cached by the extent tuple); extents are shared across cores since
slot k holds the same sequence on every core.

Host side does only data movement (gather per block_tables + layout
transforms + f32->bf16 staging); all attention math (QK^T, softmax, PV,
cache-update semantics) runs on device.

DMA layout: K and V are staged in bf16 (halves HBM traffic vs f32) and
streamed on ONE gpsimd SWDGE queue in near-sequential DRAM address order —
concurrent multi-queue streaming was measured 20% slower (two interleaved
HBM address streams defeat row locality: 296 vs 368 GB/s).  Small prologue
tensors ride the scalar HWDGE ring and the output stages the sync ring, so
neither blocks the bulk stream nor the Scalar engine's EXPs.

Tail scheduling: the PE runs in order and its clock is usually cold (1.2
GHz) by the end, so the last slabs are restructured: the final K pair is
prefetched two slabs early, V of the last two slabs is split into chunks,
and both slabs' score passes are issued before their PV chains.  After the
last V byte lands only one ~half-slab PV chunk remains on the critical
path.

Device algorithm per slab (one sequence, one kv-head, REP=4 query heads):
  - scores^T tiles  S^T[kv,r] = sum_d K[kv,d] Q[r,d]  via PE matmuls with the
    K tile as the (transposed-layout) stationary operand, accumulated in PSUM.
  - E = exp(S * scale)  on ScalarE straight out of PSUM (no max-subtraction:
    |scores| <= ~6 so fp32/bf16 exp is safe; validated 3e-3 rel err).
  - the reference overwrites cache position ctx-1 with the new token; here
    only kv < ctx-1 is loaded at all and the new token is handled separately.
  - out = (E^T @ [V | 1]) -> [4, 129]; column 128 accumulates the softmax
    denominator for free (ones column appended to V on host).
  - new token at position ctx-1: scores via one small matmul against k_new,
    exp'd, then a K=1 matmul accumulates e_new * [v_new | 1] into the same
    PSUM group.  Finally out[:, :128] * 1/out[:, 128] -> DRAM.
"""

import time

import ml_dtypes
import numpy as np

import concourse.bacc as bacc
import concourse.bass as bass
import concourse.tile as tile
from concourse import mybir
from concourse.bass_utils import run_bass_kernel_spmd

# Problem shape (hardcoded per harness contract)
B, H, KVH, D = 16, 32, 8, 128
BLOCK_SIZE = 16
MAX_BLOCKS = 256
MAX_KV = MAX_BLOCKS * BLOCK_SIZE  # 4096
SCALE = 1.0 / float(np.sqrt(D))
REP = H // KVH  # 4
N_CORES = 8
N_SLOT = B  # one slab per sequence; core c handles kv-head c of each

F32 = mybir.dt.float32
BF16 = mybir.dt.bfloat16
I32 = mybir.dt.int32

KV_TILE = 128            # kv positions per matmul tile
N_T = MAX_KV // KV_TILE  # max kv tiles per sequence (32)


def _build_kernel_body(tc, ins, outs, ext_tiles):
    nc = tc.nc
    kt = ins["kt"]        # [128, sum(ext_kv)] bf16   (d, slab-concat kv)  K^T
    vaug = ins["vaug"]    # [128, sum(n_t), 129] bf16 (p, slab-concat t, d|1)
    qt = ins["qt"]        # [128, 64] f32             (d, slot*4+r)
    ktn = ins["ktn"]      # [128, 16] f32             (d, slot)
    vnew = ins["vnew"]    # [1, 16*129] f32           slot*129 + (d|1)
    out = outs["out"]     # [4, 16, 128] f32          (r, slot, d)

    with (
        tc.tile_pool(name="singles", bufs=1) as singles,
        tc.tile_pool(name="kpool", bufs=5) as kpool,
        tc.tile_pool(name="vpool", bufs=7) as vpool,
        tc.tile_pool(name="epool", bufs=4) as epool,
        tc.tile_pool(name="opool", bufs=4) as opool,
        tc.tile_pool(name="st_ps", bufs=2, space="PSUM") as st_ps,
        tc.tile_pool(name="o_ps", bufs=4, space="PSUM") as o_ps_pool,
        tc.tile_pool(name="snew_ps", bufs=1, space="PSUM") as snew_ps_pool,
    ):
        # ---- prologue: small tensors on the scalar HWDGE ring (keeps the
        # sync ring free so the first K DMA's descgen goes out immediately),
        # DVE casts, new-token scores ----
        qtf = singles.tile([128, N_SLOT * REP], F32)
        nc.scalar.dma_start(out=qtf, in_=qt)
        qtb = singles.tile([128, N_SLOT * REP], BF16)
        nc.vector.tensor_copy(out=qtb, in_=qtf)
        ktnf = singles.tile([128, N_SLOT], F32)
        nc.scalar.dma_start(out=ktnf, in_=ktn)
        ktnb = singles.tile([128, N_SLOT], BF16)
        nc.vector.tensor_copy(out=ktnb, in_=ktnf)
        vnewf = singles.tile([1, N_SLOT * 129], F32)
        nc.scalar.dma_start(out=vnewf, in_=vnew)
        vnewb = singles.tile([1, N_SLOT * 129], BF16)
        nc.vector.tensor_copy(out=vnewb, in_=vnewf)

        # new-token scores for all slots: snew[0, k*4 + r]
        snew_ps = snew_ps_pool.tile([1, N_SLOT * REP], F32)
        for k in range(N_SLOT):
            nc.tensor.matmul(
                out=snew_ps[0:1, k * REP : (k + 1) * REP],
                lhsT=ktnb[:, k : k + 1],
                rhs=qtb[:, k * REP : (k + 1) * REP],
                start=(k == 0),
                stop=(k == N_SLOT - 1),
            )
        enew = singles.tile([1, N_SLOT * REP], BF16)
        nc.scalar.activation(
            out=enew, in_=snew_ps, func=mybir.ActivationFunctionType.Exp, scale=SCALE
        )

        # output staging in two halves so the first half's DMA ships early.
        # Staged at partitions 64-67 so the out-DMA maps to SDMA engine 1,
        # not engine 0 (engine 0 is the stream straggler: it also carries the
        # runtime's instruction-refill queue and all <=4-partition smalls).
        OBASE = 64
        ost0_full = singles.tile([OBASE + REP, N_SLOT // 2, D], F32)
        ost1_full = singles.tile([OBASE + REP, N_SLOT // 2, D], F32)
        ostages = (
            ost0_full[OBASE : OBASE + REP],
            ost1_full[OBASE : OBASE + REP],
        )

        # per-slab offsets into the concatenated DRAM staging tensors
        slab_off = []
        _ko = _vo = 0
        for kvn in ext_tiles:
            slab_off.append((_ko, _vo))
            _ko += kvn
            _vo += -(-kvn // KV_TILE)

        def slab_dims(k):
            kvn = ext_tiles[k]
            n_t = -(-kvn // KV_TILE)
            rem = kvn - (n_t - 1) * KV_TILE
            return kvn, n_t, rem

        def dma_v(k, t0, t1, eng=None):
            """DMA V tiles [t0,t1) of slab k into a fresh vtile buffer."""
            eng = eng or nc.gpsimd
            kvn, n_t, rem = slab_dims(k)
            voff = slab_off[k][1]
            nt_c = t1 - t0
            vtile = vpool.tile([128, nt_c, 129], BF16, tag="vtile")
            if t1 < n_t or rem == KV_TILE:
                eng.dma_start(
                    out=vtile, in_=vaug[:, voff + t0 : voff + t1, :]
                )
            else:
                if nt_c > 1:
                    eng.dma_start(
                        out=vtile[:, 0 : nt_c - 1, :],
                        in_=vaug[:, voff + t0 : voff + t1 - 1, :],
                    )
                eng.dma_start(
                    out=vtile[0:rem, nt_c - 1, :],
                    in_=vaug[0:rem, voff + t1 - 1, :],
                )
            return vtile

        def scores_exp(k, ktile):
            """Whole-slab scores^T then E = exp(S*scale) -> bf16 SBUF.

            scores^T: st[p, t*4 + r].  Every loaded kv row is < ctx-1 by
            construction (kvn == ctx-1), so no masking is needed anywhere.
            """
            kvn, n_t, rem = slab_dims(k)
            st = st_ps.tile([128, n_t * REP], F32, tag="st")
            # issue order puts the partial tile mid-group: the group must be
            # STARTED and STOPPED by full-128-partition matmuls or the PSUM
            # group state stays open on the uncovered partitions
            if n_t == 1:
                order = [0]
            else:
                order = [0, n_t - 1] + list(range(1, n_t - 1))
            stop_mm = None
            for i, t in enumerate(order):
                cols = KV_TILE if t < n_t - 1 else rem
                stop_mm = nc.tensor.matmul(
                    out=st[0:cols, t * REP : (t + 1) * REP],
                    lhsT=ktile[:, t * KV_TILE : t * KV_TILE + cols],
                    rhs=qtb[:, k * REP : (k + 1) * REP],
                    start=(i == 0),
                    stop=(i == len(order) - 1),
                )
            # exp in two ops so nothing reads the unwritten PSUM rows of the
            # partial last tile; the explicit dep keeps the partial read out
            # of the still-open accumulation group
            et = epool.tile([128, n_t * REP], BF16, tag="et")
            if rem == KV_TILE:
                nc.scalar.activation(
                    out=et, in_=st,
                    func=mybir.ActivationFunctionType.Exp, scale=SCALE,
                )
            else:
                if n_t > 1:
                    nc.scalar.activation(
                        out=et[:, 0 : (n_t - 1) * REP],
                        in_=st[:, 0 : (n_t - 1) * REP],
                        func=mybir.ActivationFunctionType.Exp,
                        scale=SCALE,
                    )
                e_last = nc.scalar.activation(
                    out=et[0:rem, (n_t - 1) * REP : n_t * REP],
                    in_=st[0:rem, (n_t - 1) * REP : n_t * REP],
                    func=mybir.ActivationFunctionType.Exp,
                    scale=SCALE,
                )
                tile.add_dep_helper(
                    e_last.ins, stop_mm.ins,
                    reason="partial exp after group stop",
                )
            return et

        def new_o():
            o_ps_full = o_ps_pool.tile([OBASE + REP, 129], F32, tag="o")
            return o_ps_full[OBASE : OBASE + REP]

        def pv(k, et, vtile, t0, t1, o_ps, first):
            """Accumulate E^T @ [V|1] for tiles [t0,t1) into o_ps."""
            kvn, n_t, rem = slab_dims(k)
            for j, t in enumerate(range(t0, t1)):
                kp = KV_TILE if t < n_t - 1 else rem
                nc.tensor.matmul(
                    out=o_ps,
                    lhsT=et[0:kp, t * REP : (t + 1) * REP],
                    rhs=vtile[0:kp, t - t0, :],
                    start=(first and j == 0),
                    stop=False,
                )

        def finalize(k, o_ps):
            """New-token accumulation (closes the PSUM group), then
            out = numerator * 1/denominator into the staging buffer."""
            nc.tensor.matmul(
                out=o_ps,
                lhsT=enew[0:1, k * REP : (k + 1) * REP],
                rhs=vnewb[0:1, k * 129 : (k + 1) * 129],
                start=False,
                stop=True,
            )
            recip_full = opool.tile([OBASE + REP, 1], F32, tag="recip")
            recip = recip_full[OBASE : OBASE + REP]
            nc.vector.reciprocal(out=recip, in_=o_ps[:, 128:129])
            nc.vector.tensor_scalar_mul(
                out=ostages[k // (N_SLOT // 2)][:, k % (N_SLOT // 2), :],
                in0=o_ps[:, 0:128],
                scalar1=recip,
            )

        # ---- main loop, slabs 0..N_SLOT-3: K DMA'd in slab PAIRS (adjacent
        # slabs are contiguous in DRAM and SBUF, so a pair is one long
        # per-partition run -> half the SWDGE descriptor traffic); V per
        # slab since it gates the PV tail. ----
        TAIL = 2 if N_SLOT >= 6 else 0
        n_main = N_SLOT - TAIL
        ktile_pair = None
        k_inner = 0
        tail_ktile = None
        for k in range(n_main):
            kvn, n_t, rem = slab_dims(k)
            if k % 2 == 0:
                pair_kv = kvn + (ext_tiles[k + 1] if k + 1 < n_main else 0)
                ktile_pair = kpool.tile([128, pair_kv], BF16, tag="ktile")
                # the first K pair rides the sync HWDGE ring and V0 the
                # scalar ring: both are idle at the head and HWDGE descgen
                # beats the SWDGE Q7 boot, so HBM streams ~3us sooner; the
                # bulk stays on the single SWDGE queue (concurrent bulk
                # queues measured 20% slower on HBM)
                keng = nc.sync if k == 0 else nc.gpsimd
                keng.dma_start(
                    out=ktile_pair,
                    in_=kt[:, slab_off[k][0] : slab_off[k][0] + pair_kv],
                )
                k_inner = 0
            ktile = ktile_pair[:, k_inner : k_inner + kvn]
            k_inner += kvn
            vtile = dma_v(k, 0, n_t, eng=nc.scalar if k == 0 else None)
            if TAIL and k == n_main - 2:
                # prefetch the tail K pair now: it enters the SWDGE FIFO
                # before V of the last main slabs, so both tail slabs' score
                # passes can run while the tail V chunks stream
                tail_kv = ext_tiles[n_main] + ext_tiles[n_main + 1]
                tail_ktile = kpool.tile([128, tail_kv], BF16, tag="ktile")
                nc.gpsimd.dma_start(
                    out=tail_ktile,
                    in_=kt[:, slab_off[n_main][0] : slab_off[n_main][0] + tail_kv],
                )
            et = scores_exp(k, ktile)
            o_ps = new_o()
            pv(k, et, vtile, 0, n_t, o_ps, first=True)
            finalize(k, o_ps)

        # ---- tail: both slabs' scores first (their K arrived early), then
        # the PV chains chunk-by-chunk behind the V chunk arrivals ----
        if TAIL:
            ka, kb = n_main, n_main + 1
            dims_a, dims_b = slab_dims(ka), slab_dims(kb)
            kt_a = tail_ktile[:, 0 : dims_a[0]]
            kt_b = tail_ktile[:, dims_a[0] : dims_a[0] + dims_b[0]]

            def chunks_of(n_t):
                if n_t >= 4:
                    return [(0, n_t // 2), (n_t // 2, n_t)]
                return [(0, n_t)]

            ch_a = chunks_of(dims_a[1])
            ch_b = chunks_of(dims_b[1])
            v_a = [dma_v(ka, t0, t1) for t0, t1 in ch_a]
            v_b = [dma_v(kb, t0, t1) for t0, t1 in ch_b]
            et_a = scores_exp(ka, kt_a)
            et_b = scores_exp(kb, kt_b)
            o_a, o_b = new_o(), new_o()
            for i, (t0, t1) in enumerate(ch_a):
                pv(ka, et_a, v_a[i], t0, t1, o_a, first=(i == 0))
            finalize(ka, o_a)
            for i, (t0, t1) in enumerate(ch_b):
                pv(kb, et_b, v_b[i], t0, t1, o_b, first=(i == 0))
            finalize(kb, o_b)

        # out[r, slot, d]; two DMAs (on the otherwise-idle sync ring) so the
        # first half ships mid-kernel
        half = N_SLOT // 2
        nc.sync.dma_start(out=out[:, 0:half, :], in_=ostages[0])
        nc.sync.dma_start(out=out[:, half : N_SLOT, :], in_=ostages[1])


def build_nc(ext_tiles):
    sum_kv = sum(ext_tiles)
    sum_t = sum(-(-kvn // KV_TILE) for kvn in ext_tiles)
    nc = bacc.Bacc(
        "TRN2",
        target_bir_lowering=False,
        debug=False,
        num_devices=N_CORES,
    )
    ins = {
        "kt": nc.dram_tensor(
            "kt", [128, sum_kv], BF16, kind="ExternalInput"
        ).ap(),
        "vaug": nc.dram_tensor(
            "vaug", [128, sum_t, 129], BF16, kind="ExternalInput"
        ).ap(),
        "qt": nc.dram_tensor("qt", [D, N_SLOT * REP], F32, kind="ExternalInput").ap(),
        "ktn": nc.dram_tensor("ktn", [D, N_SLOT], F32, kind="ExternalInput").ap(),
        "vnew": nc.dram_tensor(
            "vnew", [1, N_SLOT * 129], F32, kind="ExternalInput"
        ).ap(),
    }
    outs = {
        "out": nc.dram_tensor(
            "out", [REP, N_SLOT, D], F32, kind="ExternalOutput"
        ).ap(),
    }
    with tile.TileContext(nc) as tc:
        _build_kernel_body(tc, ins, outs, ext_tiles)
    nc.compile()
    return nc


def plan_assignment(context_lens):
    """Slot k holds the k-th longest-context sequence (descending, so the
    final slab — the latency tail — is the smallest).  ext_kv[k] is that
    sequence's exact valid kv count (ctx-1); identical on every core.  The
    final 128-tile of each slab is partial: only ext_kv % 128 rows are
    loaded/computed."""
    context_lens = np.asarray(context_lens)
    slot_seq = list(np.argsort(-context_lens, kind="stable").astype(int))
    ext_kv = tuple(
        min(MAX_KV, max(1, int(context_lens[s]) - 1)) for s in slot_seq
    )
    return slot_seq, ext_kv


def make_in_maps(
    q, k, v, k_cache, v_cache, block_tables, context_lens, slot_mapping,
    slot_seq, ext_tiles,
):
    """Host-side sharding: gather each sequence's blocks from the paged cache
    once, lay K out transposed (d-major) and V kv-swizzled into (partition,
    tile) order, then split by kv-head across cores.  Pure data movement; the
    ones columns are constants.  slot_mapping is implied by context_lens for
    this problem's setup (slot == position ctx-1 in the gathered view)."""
    q = np.ascontiguousarray(np.asarray(q), dtype=np.float32)
    k = np.ascontiguousarray(np.asarray(k), dtype=np.float32)
    v = np.ascontiguousarray(np.asarray(v), dtype=np.float32)
    k_cache = np.asarray(k_cache)
    v_cache = np.asarray(v_cache)
    block_tables = np.asarray(block_tables)
    context_lens = np.asarray(context_lens)

    sum_kv = sum(ext_tiles)
    sum_t = sum(-(-kvn // KV_TILE) for kvn in ext_tiles)
    # staged in bf16: halves the HBM read volume vs f32 (the kernel's PE
    # operands are bf16 anyway, so the cast costs nothing extra on device)
    kt = [np.empty((128, sum_kv), ml_dtypes.bfloat16) for _ in range(N_CORES)]
    vaug = [
        np.empty((128, sum_t, 129), ml_dtypes.bfloat16) for _ in range(N_CORES)
    ]
    koff = 0
    voff = 0
    for slot, s in enumerate(slot_seq):
        kvn = ext_tiles[slot]
        n_t = -(-kvn // KV_TILE)
        # [256 blk, 16 pos, 8 g, 128 d] -> [kv, 8, 128]
        kg = k_cache[block_tables[s]].reshape(MAX_KV, KVH, D)[:kvn]
        vg = v_cache[block_tables[s]].reshape(MAX_KV, KVH, D)[: n_t * KV_TILE]
        kT = kg.transpose(1, 2, 0)                       # [8, 128 d, kvn]
        vsw = vg.reshape(n_t, KV_TILE, KVH, D).transpose(2, 1, 0, 3)  # [8,128p,t,d]
        for c in range(N_CORES):
            kt[c][:, koff : koff + kvn] = kT[c]
            vaug[c][:, voff : voff + n_t, :D] = vsw[c]
            vaug[c][:, voff : voff + n_t, D] = 1.0
        koff += kvn
        voff += n_t

    in_maps = []
    for c in range(N_CORES):
        # q^T for this core's 4 query heads of each slot's sequence
        qt = np.ascontiguousarray(
            q[slot_seq, c * REP : (c + 1) * REP, :]      # [16, 4, 128]
            .transpose(2, 0, 1)
            .reshape(D, N_SLOT * REP)
        )
        ktn = np.ascontiguousarray(k[slot_seq, c, :].T)   # [128, 16]
        vn = np.empty((N_SLOT, 129), np.float32)
        vn[:, :D] = v[slot_seq, c, :]
        vn[:, D] = 1.0
        in_maps.append(
            dict(
                kt=kt[c],
                vaug=vaug[c],
                qt=qt,
                ktn=ktn,
                vnew=np.ascontiguousarray(vn.reshape(1, N_SLOT * 129)),
            )
        )
    return in_maps


_NC_CACHE = {}


def get_nc(ext_tiles):
    if ext_tiles not in _NC_CACHE:
        _NC_CACHE[ext_tiles] = build_nc(ext_tiles)
    return _NC_CACHE[ext_tiles]


def kernel(q, k, v, k_cache, v_cache, block_tables, context_lens, slot_mapping):
    slot_seq, ext_tiles = plan_assignment(context_lens)
    in_maps = make_in_maps(
        q, k, v, k_cache, v_cache, block_tables, context_lens, slot_mapping,
        slot_seq, ext_tiles,
    )
    nc = get_nc(ext_tiles)
    res = None
    for attempt in range(3):
        try:
            res = run_bass_kernel_spmd(nc, in_maps, core_ids=list(range(N_CORES)))
            break
        except Exception:
            # transient NRT/device hiccups recover on a fresh dispatch
            if attempt == 2:
                raise
            time.sleep(5)
    return assemble_out(
        [np.asarray(res.results[i]["out"]) for i in range(N_CORES)], slot_seq
    )


def assemble_out(core_outs, slot_seq):
    """core c's out [r, slot, d] holds head (c*4+r) of sequence slot_seq[slot]."""
    out = np.empty((B, H, D), np.float32)
    for c, co in enumerate(core_outs):
        co = co.reshape(REP, N_SLOT, D)
        for slot, s in enumerate(slot_seq):
            out[s, c * REP : (c + 1) * REP, :] = co[:, slot, :]
    return out


if __name__ == "__main__":
    nc = build_nc(tuple([N_T] * N_SLOT))
    print("build OK")


# revision 15
# speedup vs baseline: 1.0081x; 1.0081x over previous
"""Paged-attention decode (GQA, vLLM-style) on 8 TRN2 NeuronCores.

Sharding: kv-head-parallel — core c owns kv-head c (and its 4 query heads) for
ALL 16 sequences; no collectives.  Each core processes 16 slabs, one per
(sequence, head) unit, in descending context-length order.  Because a slab is
a single sequence, the graph's per-slab kv extent is exactly that sequence's
ctx-1 valid rows (the final 128-tile is partial) — invalid kv is never loaded
nor computed, which also makes any masking unnecessary.  The graph is compiled
per call (cached by the extent tuple); extents are shared across cores since
slot k holds the same sequence on every core.

Host side does only data movement (gather per block_tables + layout
transforms + f32->bf16 staging); all attention math (QK^T, softmax, PV,
cache-update semantics) runs on device.

DMA layout: K and V are staged in bf16 (halves HBM traffic vs f32) and
streamed on ONE gpsimd SWDGE queue in near-sequential DRAM address order —
concurrent multi-queue streaming was measured 20% slower (two interleaved
HBM address streams defeat row locality: 296 vs 368 GB/s).  Small prologue
tensors ride the scalar HWDGE ring and the output stages the sync ring, so
neither blocks the bulk stream nor the Scalar engine's EXPs.

Tail scheduling: the PE runs in order and its clock is usually cold (1.2
GHz) by the end, so the last slabs are restructured: the final K pair is
prefetched two slabs early, V of the last two slabs is split into chunks,
and both slabs' score passes are issued before their PV chains.  After the
last V byte lands only one ~half-slab PV chunk remains on the critical
path.

Device algorithm per slab (one sequence, one kv-head, REP=4 query heads):
  - scores^T tiles  S^T[kv,r] = sum_d K[kv,d] Q[r,d]  via PE matmuls with the
    K tile as the (transposed-layout) stationary operand, accumulated in PSUM.
  - E = exp(S * scale)  on ScalarE straight out of PSUM (no max-subtraction:
    |scores| <= ~6 so fp32/bf16 exp is safe; validated 3e-3 rel err).
  - the reference overwrites cache position ctx-1 with the new token; here
    only kv < ctx-1 is loaded at all and the new token is handled separately.
  - out = (E^T @ [V | 1]) -> [4, 129]; column 128 accumulates the softmax
    denominator for free (ones column appended to V on host).
  - new token at position ctx-1: scores via one small matmul against k_new,
    exp'd, then a K=1 matmul accumulates e_new * [v_new | 1] into the same
    PSUM group.  Finally out[:, :128] * 1/out[:, 128] -> DRAM.
"""

import time

import ml_dtypes
import numpy as np

import concourse.bacc as bacc
import concourse.bass as bass
import concourse.tile as tile
from concourse import mybir
from concourse.bass_utils import run_bass_kernel_spmd

# Problem shape (hardcoded per harness contract)
B, H, KVH, D = 16, 32, 8, 128
BLOCK_SIZE = 16
MAX_BLOCKS = 256
MAX_KV = MAX_BLOCKS * BLOCK_SIZE  # 4096
SCALE = 1.0 / float(np.sqrt(D))
REP = H // KVH  # 4
N_CORES = 8
N_SLOT = B  # one slab per sequence; core c handles kv-head c of each

F32 = mybir.dt.float32
BF16 = mybir.dt.bfloat16
I32 = mybir.dt.int32

KV_TILE = 128            # kv positions per matmul tile
N_T = MAX_KV // KV_TILE  # max kv tiles per sequence (32)


def _build_kernel_body(tc, ins, outs, ext_tiles):
    nc = tc.nc
    kt = ins["kt"]        # [128, sum(ext_kv)] bf16   (d, slab-concat kv)  K^T
    vaug = ins["vaug"]    # [128, sum(n_t), 129] bf16 (p, slab-concat t, d|1)
    qt = ins["qt"]        # [128, 64] f32             (d, slot*4+r)
    ktn = ins["ktn"]      # [128, 16] f32             (d, slot)
    vnew = ins["vnew"]    # [1, 16*129] f32           slot*129 + (d|1)
    out = outs["out"]     # [4, 16, 128] f32          (r, slot, d)

    with (
        tc.tile_pool(name="singles", bufs=1) as singles,
        tc.tile_pool(name="kpool", bufs=5) as kpool,
        tc.tile_pool(name="vpool", bufs=7) as vpool,
        tc.tile_pool(name="epool", bufs=4) as epool,
        tc.tile_pool(name="opool", bufs=4) as opool,
        tc.tile_pool(name="st_ps", bufs=2, space="PSUM") as st_ps,
        tc.tile_pool(name="o_ps", bufs=4, space="PSUM") as o_ps_pool,
        tc.tile_pool(name="snew_ps", bufs=1, space="PSUM") as snew_ps_pool,
    ):
        # ---- prologue: small tensors on the scalar HWDGE ring (keeps the
        # sync ring free so the first K DMA's descgen goes out immediately),
        # DVE casts, new-token scores ----
        qtf = singles.tile([128, N_SLOT * REP], F32)
        nc.scalar.dma_start(out=qtf, in_=qt)
        qtb = singles.tile([128, N_SLOT * REP], BF16)
        nc.vector.tensor_copy(out=qtb, in_=qtf)
        ktnf = singles.tile([128, N_SLOT], F32)
        nc.scalar.dma_start(out=ktnf, in_=ktn)
        ktnb = singles.tile([128, N_SLOT], BF16)
        nc.vector.tensor_copy(out=ktnb, in_=ktnf)
        vnewf = singles.tile([1, N_SLOT * 129], F32)
        nc.scalar.dma_start(out=vnewf, in_=vnew)
        vnewb = singles.tile([1, N_SLOT * 129], BF16)
        nc.vector.tensor_copy(out=vnewb, in_=vnewf)

        # new-token scores for all slots: snew[0, k*4 + r]
        snew_ps = snew_ps_pool.tile([1, N_SLOT * REP], F32)
        for k in range(N_SLOT):
            nc.tensor.matmul(
                out=snew_ps[0:1, k * REP : (k + 1) * REP],
                lhsT=ktnb[:, k : k + 1],
                rhs=qtb[:, k * REP : (k + 1) * REP],
                start=(k == 0),
                stop=(k == N_SLOT - 1),
            )
        enew = singles.tile([1, N_SLOT * REP], BF16)
        nc.scalar.activation(
            out=enew, in_=snew_ps, func=mybir.ActivationFunctionType.Exp, scale=SCALE
        )

        # output staging in two halves so the first half's DMA ships early.
        # Staged at partitions 64-67 so the out-DMA maps to SDMA engine 1,
        # not engine 0 (engine 0 is the stream straggler: it also carries the
        # runtime's instruction-refill queue and all <=4-partition smalls).
        OBASE = 64
        ost0_full = singles.tile([OBASE + REP, N_SLOT // 2, D], F32)
        ost1_full = singles.tile([OBASE + REP, N_SLOT // 2, D], F32)
        ostages = (
            ost0_full[OBASE : OBASE + REP],
            ost1_full[OBASE : OBASE + REP],
        )

        # per-slab offsets into the concatenated DRAM staging tensors
        slab_off = []
        _ko = _vo = 0
        for kvn in ext_tiles:
            slab_off.append((_ko, _vo))
            _ko += kvn
            _vo += -(-kvn // KV_TILE)

        def slab_dims(k):
            kvn = ext_tiles[k]
            n_t = -(-kvn // KV_TILE)
            rem = kvn - (n_t - 1) * KV_TILE
            return kvn, n_t, rem

        def dma_v(k, t0, t1, eng=None):
            """DMA V tiles [t0,t1) of slab k into a fresh vtile buffer."""
            eng = eng or nc.gpsimd
            kvn, n_t, rem = slab_dims(k)
            voff = slab_off[k][1]
            nt_c = t1 - t0
            vtile = vpool.tile([128, nt_c, 129], BF16, tag="vtile")
            if t1 < n_t or rem == KV_TILE:
                eng.dma_start(
                    out=vtile, in_=vaug[:, voff + t0 : voff + t1, :]
                )
            else:
                if nt_c > 1:
                    eng.dma_start(
                        out=vtile[:, 0 : nt_c - 1, :],
                        in_=vaug[:, voff + t0 : voff + t1 - 1, :],
                    )
                eng.dma_start(
                    out=vtile[0:rem, nt_c - 1, :],
                    in_=vaug[0:rem, voff + t1 - 1, :],
                )
            return vtile

        def score_ops(k, ktile):
            """Whole-slab scores^T as a list of per-tile closures, plus a
            finisher emitting E = exp(S*scale) -> bf16 SBUF.  Split this way
            so a caller can interleave the score matmuls with another slab's
            PV matmuls (the 128-row score LDWEIGHTS then hides under the
            other slab's 129-column PV matmul on the in-order PE).

            scores^T: st[p, t*4 + r].  Every loaded kv row is < ctx-1 by
            construction (kvn == ctx-1), so no masking is needed anywhere.
            """
            kvn, n_t, rem = slab_dims(k)
            st = st_ps.tile([128, n_t * REP], F32, tag="st")
            # issue order puts the partial tile mid-group: the group must be
            # STARTED and STOPPED by full-128-partition matmuls or the PSUM
            # group state stays open on the uncovered partitions
            if n_t == 1:
                order = [0]
            else:
                order = [0, n_t - 1] + list(range(1, n_t - 1))
            state = {}

            def mk(i, t):
                def op():
                    mm = nc.tensor.matmul(
                        out=st[0 : (KV_TILE if t < n_t - 1 else rem),
                               t * REP : (t + 1) * REP],
                        lhsT=ktile[:, t * KV_TILE : t * KV_TILE
                                   + (KV_TILE if t < n_t - 1 else rem)],
                        rhs=qtb[:, k * REP : (k + 1) * REP],
                        start=(i == 0),
                        stop=(i == len(order) - 1),
                    )
                    if i == len(order) - 1:
                        state["stop_mm"] = mm
                return op

            ops = [mk(i, t) for i, t in enumerate(order)]

            def finish():
                # exp in two ops so nothing reads the unwritten PSUM rows of
                # the partial last tile; the explicit dep keeps the partial
                # read out of the still-open accumulation group
                et = epool.tile([128, n_t * REP], BF16, tag="et")
                if rem == KV_TILE:
                    nc.scalar.activation(
                        out=et, in_=st,
                        func=mybir.ActivationFunctionType.Exp, scale=SCALE,
                    )
                else:
                    if n_t > 1:
                        nc.scalar.activation(
                            out=et[:, 0 : (n_t - 1) * REP],
                            in_=st[:, 0 : (n_t - 1) * REP],
                            func=mybir.ActivationFunctionType.Exp,
                            scale=SCALE,
                        )
                    e_last = nc.scalar.activation(
                        out=et[0:rem, (n_t - 1) * REP : n_t * REP],
                        in_=st[0:rem, (n_t - 1) * REP : n_t * REP],
                        func=mybir.ActivationFunctionType.Exp,
                        scale=SCALE,
                    )
                    tile.add_dep_helper(
                        e_last.ins, state["stop_mm"].ins,
                        reason="partial exp after group stop",
                    )
                return et

            return ops, finish

        def scores_exp(k, ktile):
            ops, finish = score_ops(k, ktile)
            for op in ops:
                op()
            return finish()

        def new_o():
            o_ps_full = o_ps_pool.tile([OBASE + REP, 129], F32, tag="o")
            return o_ps_full[OBASE : OBASE + REP]

        def pv(k, et, vtile, t0, t1, o_ps, first, vbase=None):
            """Accumulate E^T @ [V|1] for tiles [t0,t1) into o_ps.  vbase is
            the slab tile index at vtile's first entry (defaults to t0, i.e.
            a chunk-local buffer starting at t0)."""
            kvn, n_t, rem = slab_dims(k)
            if vbase is None:
                vbase = t0
            for j, t in enumerate(range(t0, t1)):
                kp = KV_TILE if t < n_t - 1 else rem
                nc.tensor.matmul(
                    out=o_ps,
                    lhsT=et[0:kp, t * REP : (t + 1) * REP],
                    rhs=vtile[0:kp, t - vbase, :],
                    start=(first and j == 0),
                    stop=False,
                )

        def finalize(k, o_ps):
            """New-token accumulation (closes the PSUM group), then
            out = numerator * 1/denominator into the staging buffer."""
            nc.tensor.matmul(
                out=o_ps,
                lhsT=enew[0:1, k * REP : (k + 1) * REP],
                rhs=vnewb[0:1, k * 129 : (k + 1) * 129],
                start=False,
                stop=True,
            )
            recip_full = opool.tile([OBASE + REP, 1], F32, tag="recip")
            recip = recip_full[OBASE : OBASE + REP]
            nc.vector.reciprocal(out=recip, in_=o_ps[:, 128:129])
            nc.vector.tensor_scalar_mul(
                out=ostages[k // (N_SLOT // 2)][:, k % (N_SLOT // 2), :],
                in0=o_ps[:, 0:128],
                scalar1=recip,
            )

        # ---- main loop, slab PAIRS (adjacent slabs are contiguous in DRAM
        # and SBUF, so a K pair is one long per-partition run -> half the
        # SWDGE descriptor traffic); V per slab since it gates the PV tail.
        # Within a pair the second slab's score matmuls interleave with the
        # first slab's PV matmuls: the score LDWEIGHTS (128 rows) hides
        # under the PV matmul (129 columns), cutting PE time per pair ~20%
        # so a cold-clocked PE (1.2 GHz HAM state) still keeps up with the
        # HBM stream instead of back-pressuring it. ----
        TAIL = 2 if N_SLOT >= 6 else 0
        n_main = N_SLOT - TAIL
        tail_ktile = None
        for k in range(0, n_main, 2):
            kvn, n_t, rem = slab_dims(k)
            has_b = k + 1 < n_main
            pair_kv = kvn + (ext_tiles[k + 1] if has_b else 0)
            ktile_pair = kpool.tile([128, pair_kv], BF16, tag="ktile")
            nc.gpsimd.dma_start(
                out=ktile_pair,
                in_=kt[:, slab_off[k][0] : slab_off[k][0] + pair_kv],
            )
            vtile = dma_v(k, 0, n_t)
            if TAIL and k == n_main - 2:
                # prefetch the tail K pair now: it enters the SWDGE FIFO
                # before V of the last main slabs, so both tail slabs' score
                # passes can run while the tail V chunks stream
                tail_kv = ext_tiles[n_main] + ext_tiles[n_main + 1]
                tail_ktile = kpool.tile([128, tail_kv], BF16, tag="ktile")
                nc.gpsimd.dma_start(
                    out=tail_ktile,
                    in_=kt[:, slab_off[n_main][0] : slab_off[n_main][0] + tail_kv],
                )
            vtile_b = dma_v(k + 1, 0, slab_dims(k + 1)[1]) if has_b else None

            et = scores_exp(k, ktile_pair[:, 0:kvn])
            o_ps = new_o()
            if not has_b:
                pv(k, et, vtile, 0, n_t, o_ps, first=True)
                finalize(k, o_ps)
                continue
            kvn_b, n_t_b, rem_b = slab_dims(k + 1)
            ops_b, finish_b = score_ops(
                k + 1, ktile_pair[:, kvn : kvn + kvn_b]
            )
            # a few score matmuls go first to cover the ScalarE exp latency
            # the first PV matmul waits on; after that, alternate PV/score
            PRE = min(3, len(ops_b))
            for i in range(PRE):
                ops_b[i]()
            nxt = PRE
            for t in range(n_t):
                pv(k, et, vtile, t, t + 1, o_ps, first=(t == 0), vbase=0)
                if nxt < len(ops_b):
                    ops_b[nxt]()
                    nxt += 1
            while nxt < len(ops_b):
                ops_b[nxt]()
                nxt += 1
            et_b = finish_b()
            finalize(k, o_ps)
            o_ps_b = new_o()
            pv(k + 1, et_b, vtile_b, 0, n_t_b, o_ps_b, first=True)
            finalize(k + 1, o_ps_b)

        # ---- tail: both slabs' scores first (their K arrived early), then
        # the PV chains chunk-by-chunk behind the V chunk arrivals ----
        if TAIL:
            ka, kb = n_main, n_main + 1
            dims_a, dims_b = slab_dims(ka), slab_dims(kb)
            kt_a = tail_ktile[:, 0 : dims_a[0]]
            kt_b = tail_ktile[:, dims_a[0] : dims_a[0] + dims_b[0]]

            def chunks_of(n_t):
                if n_t >= 4:
                    return [(0, n_t // 2), (n_t // 2, n_t)]
                return [(0, n_t)]

            ch_a = chunks_of(dims_a[1])
            ch_b = chunks_of(dims_b[1])
            v_a = [dma_v(ka, t0, t1) for t0, t1 in ch_a]
            v_b = [dma_v(kb, t0, t1) for t0, t1 in ch_b]
            et_a = scores_exp(ka, kt_a)
            et_b = scores_exp(kb, kt_b)
            o_a, o_b = new_o(), new_o()
            for i, (t0, t1) in enumerate(ch_a):
                pv(ka, et_a, v_a[i], t0, t1, o_a, first=(i == 0))
            finalize(ka, o_a)
            for i, (t0, t1) in enumerate(ch_b):
                pv(kb, et_b, v_b[i], t0, t1, o_b, first=(i == 0))
            finalize(kb, o_b)

        # out[r, slot, d]; two DMAs (on the otherwise-idle sync ring) so the
        # first half ships mid-kernel
        half = N_SLOT // 2
        nc.sync.dma_start(out=out[:, 0:half, :], in_=ostages[0])
        nc.sync.dma_start(out=out[:, half : N_SLOT, :], in_=ostages[1])


def build_nc(ext_tiles):
    sum_kv = sum(ext_tiles)
    sum_t = sum(-(-kvn // KV_TILE) for kvn in ext_tiles)
    nc = bacc.Bacc(
        "TRN2",
        target_bir_lowering=False,
        debug=False,
        num_devices=N_CORES,
    )
    ins = {
        "kt": nc.dram_tensor(
            "kt", [128, sum_kv], BF16, kind="ExternalInput"
        ).ap(),
        "vaug": nc.dram_tensor(
            "vaug", [128, sum_t, 129], BF16, kind="ExternalInput"
        ).ap(),
        "qt": nc.dram_tensor("qt", [D, N_SLOT * REP], F32, kind="ExternalInput").ap(),
        "ktn": nc.dram_tensor("ktn", [D, N_SLOT], F32, kind="ExternalInput").ap(),
        "vnew": nc.dram_tensor(
            "vnew", [1, N_SLOT * 129], F32, kind="ExternalInput"
        ).ap(),
    }
    outs = {
        "out": nc.dram_tensor(
            "out", [REP, N_SLOT, D], F32, kind="ExternalOutput"
        ).ap(),
    }
    with tile.TileContext(nc) as tc:
        _build_kernel_body(tc, ins, outs, ext_tiles)
    nc.compile()
    return nc


def plan_assignment(context_lens):
    """Slot k holds the k-th longest-context sequence (descending, so the
    final slab — the latency tail — is the smallest).  ext_kv[k] is that
    sequence's exact valid kv count (ctx-1); identical on every core.  The
    final 128-tile of each slab is partial: only ext_kv % 128 rows are
    loaded/computed."""
    context_lens = np.asarray(context_lens)
    slot_seq = list(np.argsort(-context_lens, kind="stable").astype(int))
    ext_kv = tuple(
        min(MAX_KV, max(1, int(context_lens[s]) - 1)) for s in slot_seq
    )
    return slot_seq, ext_kv


def make_in_maps(
    q, k, v, k_cache, v_cache, block_tables, context_lens, slot_mapping,
    slot_seq, ext_tiles,
):
    """Host-side sharding: gather each sequence's blocks from the paged cache
    once, lay K out transposed (d-major) and V kv-swizzled into (partition,
    tile) order, then split by kv-head across cores.  Pure data movement; the
    ones columns are constants.  slot_mapping is implied by context_lens for
    this problem's setup (slot == position ctx-1 in the gathered view)."""
    q = np.ascontiguousarray(np.asarray(q), dtype=np.float32)
    k = np.ascontiguousarray(np.asarray(k), dtype=np.float32)
    v = np.ascontiguousarray(np.asarray(v), dtype=np.float32)
    k_cache = np.asarray(k_cache)
    v_cache = np.asarray(v_cache)
    block_tables = np.asarray(block_tables)
    context_lens = np.asarray(context_lens)

    sum_kv = sum(ext_tiles)
    sum_t = sum(-(-kvn // KV_TILE) for kvn in ext_tiles)
    # staged in bf16: halves the HBM read volume vs f32 (the kernel's PE
    # operands are bf16 anyway, so the cast costs nothing extra on device)
    kt = [np.empty((128, sum_kv), ml_dtypes.bfloat16) for _ in range(N_CORES)]
    vaug = [
        np.empty((128, sum_t, 129), ml_dtypes.bfloat16) for _ in range(N_CORES)
    ]
    koff = 0
    voff = 0
    for slot, s in enumerate(slot_seq):
        kvn = ext_tiles[slot]
        n_t = -(-kvn // KV_TILE)
        # [256 blk, 16 pos, 8 g, 128 d] -> [kv, 8, 128]
        kg = k_cache[block_tables[s]].reshape(MAX_KV, KVH, D)[:kvn]
        vg = v_cache[block_tables[s]].reshape(MAX_KV, KVH, D)[: n_t * KV_TILE]
        kT = kg.transpose(1, 2, 0)                       # [8, 128 d, kvn]
        vsw = vg.reshape(n_t, KV_TILE, KVH, D).transpose(2, 1, 0, 3)  # [8,128p,t,d]
        for c in range(N_CORES):
            kt[c][:, koff : koff + kvn] = kT[c]
            vaug[c][:, voff : voff + n_t, :D] = vsw[c]
            vaug[c][:, voff : voff + n_t, D] = 1.0
        koff += kvn
        voff += n_t

    in_maps = []
    for c in range(N_CORES):
        # q^T for this core's 4 query heads of each slot's sequence
        qt = np.ascontiguousarray(
            q[slot_seq, c * REP : (c + 1) * REP, :]      # [16, 4, 128]
            .transpose(2, 0, 1)
            .reshape(D, N_SLOT * REP)
        )
        ktn = np.ascontiguousarray(k[slot_seq, c, :].T)   # [128, 16]
        vn = np.empty((N_SLOT, 129), np.float32)
        vn[:, :D] = v[slot_seq, c, :]
        vn[:, D] = 1.0
        in_maps.append(
            dict(
                kt=kt[c],
                vaug=vaug[c],
                qt=qt,
                ktn=ktn,
                vnew=np.ascontiguousarray(vn.reshape(1, N_SLOT * 129)),
            )
        )
    return in_maps


_NC_CACHE = {}


def get_nc(ext_tiles):
    if ext_tiles not in _NC_CACHE:
        _NC_CACHE[ext_tiles] = build_nc(ext_tiles)
    return _NC_CACHE[ext_tiles]


def kernel(q, k, v, k_cache, v_cache, block_tables, context_lens, slot_mapping):
    slot_seq, ext_tiles = plan_assignment(context_lens)
    in_maps = make_in_maps(
        q, k, v, k_cache, v_cache, block_tables, context_lens, slot_mapping,
        slot_seq, ext_tiles,
    )
    nc = get_nc(ext_tiles)
    res = None
    for attempt in range(3):
        try:
            res = run_bass_kernel_spmd(nc, in_maps, core_ids=list(range(N_CORES)))
            break
        except Exception:
            # transient NRT/device hiccups recover on a fresh dispatch
            if attempt == 2:
                raise
            time.sleep(5)
    return assemble_out(
        [np.asarray(res.results[i]["out"]) for i in range(N_CORES)], slot_seq
    )


def assemble_out(core_outs, slot_seq):
    """core c's out [r, slot, d] holds head (c*4+r) of sequence slot_seq[slot]."""
    out = np.empty((B, H, D), np.float32)
    for c, co in enumerate(core_outs):
        co = co.reshape(REP, N_SLOT, D)
        for slot, s in enumerate(slot_seq):
            out[s, c * REP : (c + 1) * REP, :] = co[:, slot, :]
    return out


if __name__ == "__main__":
    nc = build_nc(tuple([N_T] * N_SLOT))
    print("build OK")


# revision 16
# speedup vs baseline: 1.0107x; 1.0026x over previous
"""Paged-attention decode (GQA, vLLM-style) on 8 TRN2 NeuronCores.

Sharding: kv-head-parallel — core c owns kv-head c (and its 4 query heads) for
ALL 16 sequences; no collectives.  Each core processes 16 slabs, one per
(sequence, head) unit, in descending context-length order.  Because a slab is
a single sequence, the graph's per-slab kv extent is exactly that sequence's
ctx-1 valid rows (the final 128-tile is partial) — invalid kv is never loaded
nor computed, which also makes any masking unnecessary.  The graph is compiled
per call (cached by the extent tuple); extents are shared across cores since
slot k holds the same sequence on every core.

Host side does only data movement (gather per block_tables + layout
transforms + f32->bf16 staging); all attention math (QK^T, softmax, PV,
cache-update semantics) runs on device.

DMA layout: K and V are staged in bf16 (halves HBM traffic vs f32) and
streamed on ONE gpsimd SWDGE queue in near-sequential DRAM address order —
concurrent multi-queue streaming was measured 20% slower (two interleaved
HBM address streams defeat row locality: 296 vs 368 GB/s).  Small prologue
tensors ride the scalar HWDGE ring and the output stages the sync ring, so
neither blocks the bulk stream nor the Scalar engine's EXPs.

Tail scheduling: the PE runs in order and its clock is usually cold (1.2
GHz) by the end, so the last slabs are restructured: the final K pair is
prefetched two slabs early, V of the last two slabs is split into chunks,
and both slabs' score passes are issued before their PV chains.  After the
last V byte lands only one ~half-slab PV chunk remains on the critical
path.

Device algorithm per slab (one sequence, one kv-head, REP=4 query heads):
  - scores^T tiles  S^T[kv,r] = sum_d K[kv,d] Q[r,d]  via PE matmuls with the
    K tile as the (transposed-layout) stationary operand, accumulated in PSUM.
  - E = exp(S * scale)  on ScalarE straight out of PSUM (no max-subtraction:
    |scores| <= ~6 so fp32/bf16 exp is safe; validated 3e-3 rel err).
  - the reference overwrites cache position ctx-1 with the new token; here
    only kv < ctx-1 is loaded at all and the new token is handled separately.
  - out = (E^T @ [V | 1]) -> [4, 129]; column 128 accumulates the softmax
    denominator for free (ones column appended to V on host).
  - new token at position ctx-1: scores via one small matmul against k_new,
    exp'd, then a K=1 matmul accumulates e_new * [v_new | 1] into the same
    PSUM group.  Finally out[:, :128] * 1/out[:, 128] -> DRAM.
"""

import time

import ml_dtypes
import numpy as np

import concourse.bacc as bacc
import concourse.bass as bass
import concourse.tile as tile
from concourse import mybir
from concourse.bass_utils import run_bass_kernel_spmd

# Problem shape (hardcoded per harness contract)
B, H, KVH, D = 16, 32, 8, 128
BLOCK_SIZE = 16
MAX_BLOCKS = 256
MAX_KV = MAX_BLOCKS * BLOCK_SIZE  # 4096
SCALE = 1.0 / float(np.sqrt(D))
REP = H // KVH  # 4
N_CORES = 8
N_SLOT = B  # one slab per sequence; core c handles kv-head c of each

F32 = mybir.dt.float32
BF16 = mybir.dt.bfloat16
I32 = mybir.dt.int32

KV_TILE = 128            # kv positions per matmul tile
N_T = MAX_KV // KV_TILE  # max kv tiles per sequence (32)


def _build_kernel_body(tc, ins, outs, ext_tiles):
    nc = tc.nc
    kt = ins["kt"]        # [128, sum(ext_kv)] bf16   (d, slab-concat kv)  K^T
    vaug = ins["vaug"]    # [128, sum(n_t), 129] bf16 (p, slab-concat t, d|1)
    qt = ins["qt"]        # [128, 64] f32             (d, slot*4+r)
    ktn = ins["ktn"]      # [128, 16] f32             (d, slot)
    vnew = ins["vnew"]    # [1, 16*129] f32           slot*129 + (d|1)
    out = outs["out"]     # [4, 16, 128] f32          (r, slot, d)

    with (
        tc.tile_pool(name="singles", bufs=1) as singles,
        tc.tile_pool(name="kpool", bufs=5) as kpool,
        tc.tile_pool(name="vpool", bufs=7) as vpool,
        tc.tile_pool(name="epool", bufs=4) as epool,
        tc.tile_pool(name="opool", bufs=4) as opool,
        tc.tile_pool(name="st_ps", bufs=2, space="PSUM") as st_ps,
        tc.tile_pool(name="o_ps", bufs=4, space="PSUM") as o_ps_pool,
        tc.tile_pool(name="snew_ps", bufs=1, space="PSUM") as snew_ps_pool,
    ):
        # ---- prologue: small tensors on the scalar HWDGE ring (keeps the
        # sync ring free so the first K DMA's descgen goes out immediately),
        # DVE casts, new-token scores ----
        qtf = singles.tile([128, N_SLOT * REP], F32)
        nc.scalar.dma_start(out=qtf, in_=qt)
        qtb = singles.tile([128, N_SLOT * REP], BF16)
        nc.vector.tensor_copy(out=qtb, in_=qtf)
        ktnf = singles.tile([128, N_SLOT], F32)
        nc.scalar.dma_start(out=ktnf, in_=ktn)
        ktnb = singles.tile([128, N_SLOT], BF16)
        nc.vector.tensor_copy(out=ktnb, in_=ktnf)
        vnewf = singles.tile([1, N_SLOT * 129], F32)
        nc.scalar.dma_start(out=vnewf, in_=vnew)
        vnewb = singles.tile([1, N_SLOT * 129], BF16)
        nc.vector.tensor_copy(out=vnewb, in_=vnewf)

        # new-token scores for all slots: snew[0, k*4 + r]
        snew_ps = snew_ps_pool.tile([1, N_SLOT * REP], F32)
        for k in range(N_SLOT):
            nc.tensor.matmul(
                out=snew_ps[0:1, k * REP : (k + 1) * REP],
                lhsT=ktnb[:, k : k + 1],
                rhs=qtb[:, k * REP : (k + 1) * REP],
                start=(k == 0),
                stop=(k == N_SLOT - 1),
            )
        enew = singles.tile([1, N_SLOT * REP], BF16)
        nc.scalar.activation(
            out=enew, in_=snew_ps, func=mybir.ActivationFunctionType.Exp, scale=SCALE
        )

        # output staging in two halves so the first half's DMA ships early.
        # Staged at partitions 64-67 so the out-DMA maps to SDMA engine 1,
        # not engine 0 (engine 0 is the stream straggler: it also carries the
        # runtime's instruction-refill queue and all <=4-partition smalls).
        OBASE = 64
        ost0_full = singles.tile([OBASE + REP, N_SLOT // 2, D], F32)
        ost1_full = singles.tile([OBASE + REP, N_SLOT // 2, D], F32)
        ostages = (
            ost0_full[OBASE : OBASE + REP],
            ost1_full[OBASE : OBASE + REP],
        )

        # per-slab offsets into the concatenated DRAM staging tensors
        slab_off = []
        _ko = _vo = 0
        for kvn in ext_tiles:
            slab_off.append((_ko, _vo))
            _ko += kvn
            _vo += -(-kvn // KV_TILE)

        def slab_dims(k):
            kvn = ext_tiles[k]
            n_t = -(-kvn // KV_TILE)
            rem = kvn - (n_t - 1) * KV_TILE
            return kvn, n_t, rem

        def dma_v(k, t0, t1, eng=None):
            """DMA V tiles [t0,t1) of slab k into a fresh vtile buffer."""
            eng = eng or nc.gpsimd
            kvn, n_t, rem = slab_dims(k)
            voff = slab_off[k][1]
            nt_c = t1 - t0
            vtile = vpool.tile([128, nt_c, 129], BF16, tag="vtile")
            if t1 < n_t or rem == KV_TILE:
                eng.dma_start(
                    out=vtile, in_=vaug[:, voff + t0 : voff + t1, :]
                )
            else:
                if nt_c > 1:
                    eng.dma_start(
                        out=vtile[:, 0 : nt_c - 1, :],
                        in_=vaug[:, voff + t0 : voff + t1 - 1, :],
                    )
                eng.dma_start(
                    out=vtile[0:rem, nt_c - 1, :],
                    in_=vaug[0:rem, voff + t1 - 1, :],
                )
            return vtile

        def score_ops(k, ktile):
            """Whole-slab scores^T as a list of per-tile closures, plus a
            finisher emitting E = exp(S*scale) -> bf16 SBUF.  Split this way
            so a caller can interleave the score matmuls with another slab's
            PV matmuls (the 128-row score LDWEIGHTS then hides under the
            other slab's 129-column PV matmul on the in-order PE).

            scores^T: st[p, t*4 + r].  Every loaded kv row is < ctx-1 by
            construction (kvn == ctx-1), so no masking is needed anywhere.
            """
            kvn, n_t, rem = slab_dims(k)
            st = st_ps.tile([128, n_t * REP], F32, tag="st")
            # issue order puts the partial tile mid-group: the group must be
            # STARTED and STOPPED by full-128-partition matmuls or the PSUM
            # group state stays open on the uncovered partitions
            if n_t == 1:
                order = [0]
            else:
                order = [0, n_t - 1] + list(range(1, n_t - 1))
            state = {}

            def mk(i, t):
                def op():
                    mm = nc.tensor.matmul(
                        out=st[0 : (KV_TILE if t < n_t - 1 else rem),
                               t * REP : (t + 1) * REP],
                        lhsT=ktile[:, t * KV_TILE : t * KV_TILE
                                   + (KV_TILE if t < n_t - 1 else rem)],
                        rhs=qtb[:, k * REP : (k + 1) * REP],
                        start=(i == 0),
                        stop=(i == len(order) - 1),
                    )
                    if i == len(order) - 1:
                        state["stop_mm"] = mm
                return op

            ops = [mk(i, t) for i, t in enumerate(order)]

            def finish():
                # exp in two ops so nothing reads the unwritten PSUM rows of
                # the partial last tile; the explicit dep keeps the partial
                # read out of the still-open accumulation group
                et = epool.tile([128, n_t * REP], BF16, tag="et")
                if rem == KV_TILE:
                    nc.scalar.activation(
                        out=et, in_=st,
                        func=mybir.ActivationFunctionType.Exp, scale=SCALE,
                    )
                else:
                    if n_t > 1:
                        nc.scalar.activation(
                            out=et[:, 0 : (n_t - 1) * REP],
                            in_=st[:, 0 : (n_t - 1) * REP],
                            func=mybir.ActivationFunctionType.Exp,
                            scale=SCALE,
                        )
                    e_last = nc.scalar.activation(
                        out=et[0:rem, (n_t - 1) * REP : n_t * REP],
                        in_=st[0:rem, (n_t - 1) * REP : n_t * REP],
                        func=mybir.ActivationFunctionType.Exp,
                        scale=SCALE,
                    )
                    tile.add_dep_helper(
                        e_last.ins, state["stop_mm"].ins,
                        reason="partial exp after group stop",
                    )
                return et

            return ops, finish

        def scores_exp(k, ktile):
            ops, finish = score_ops(k, ktile)
            for op in ops:
                op()
            return finish()

        def new_o():
            o_ps_full = o_ps_pool.tile([OBASE + REP, 129], F32, tag="o")
            return o_ps_full[OBASE : OBASE + REP]

        def pv(k, et, vtile, t0, t1, o_ps, first, vbase=None):
            """Accumulate E^T @ [V|1] for tiles [t0,t1) into o_ps.  vbase is
            the slab tile index at vtile's first entry (defaults to t0, i.e.
            a chunk-local buffer starting at t0)."""
            kvn, n_t, rem = slab_dims(k)
            if vbase is None:
                vbase = t0
            for j, t in enumerate(range(t0, t1)):
                kp = KV_TILE if t < n_t - 1 else rem
                nc.tensor.matmul(
                    out=o_ps,
                    lhsT=et[0:kp, t * REP : (t + 1) * REP],
                    rhs=vtile[0:kp, t - vbase, :],
                    start=(first and j == 0),
                    stop=False,
                )

        def finalize(k, o_ps):
            """New-token accumulation (closes the PSUM group), then
            out = numerator * 1/denominator into the staging buffer."""
            nc.tensor.matmul(
                out=o_ps,
                lhsT=enew[0:1, k * REP : (k + 1) * REP],
                rhs=vnewb[0:1, k * 129 : (k + 1) * 129],
                start=False,
                stop=True,
            )
            recip_full = opool.tile([OBASE + REP, 1], F32, tag="recip")
            recip = recip_full[OBASE : OBASE + REP]
            nc.vector.reciprocal(out=recip, in_=o_ps[:, 128:129])
            nc.vector.tensor_scalar_mul(
                out=ostages[k // (N_SLOT // 2)][:, k % (N_SLOT // 2), :],
                in0=o_ps[:, 0:128],
                scalar1=recip,
            )

        # ---- main loop, slab PAIRS (adjacent slabs are contiguous in DRAM
        # and SBUF, so a K pair is one long per-partition run -> half the
        # SWDGE descriptor traffic); V per slab since it gates the PV tail.
        # Within a pair the second slab's score matmuls interleave with the
        # first slab's PV matmuls: the score LDWEIGHTS (128 rows) hides
        # under the PV matmul (129 columns), cutting PE time per pair ~20%
        # so a cold-clocked PE (1.2 GHz HAM state) still keeps up with the
        # HBM stream instead of back-pressuring it. ----
        TAIL = 2 if N_SLOT >= 6 else 0
        n_main = N_SLOT - TAIL
        tail_ktile = None
        for k in range(0, n_main, 2):
            kvn, n_t, rem = slab_dims(k)
            has_b = k + 1 < n_main
            pair_kv = kvn + (ext_tiles[k + 1] if has_b else 0)
            ktile_pair = kpool.tile([128, pair_kv], BF16, tag="ktile")
            nc.gpsimd.dma_start(
                out=ktile_pair,
                in_=kt[:, slab_off[k][0] : slab_off[k][0] + pair_kv],
            )
            vtile = dma_v(k, 0, n_t)
            if TAIL and k == n_main - 2:
                # prefetch the tail K pair now: it enters the SWDGE FIFO
                # before V of the last main slabs, so both tail slabs' score
                # passes can run while the tail V chunks stream
                tail_kv = ext_tiles[n_main] + ext_tiles[n_main + 1]
                tail_ktile = kpool.tile([128, tail_kv], BF16, tag="ktile")
                nc.gpsimd.dma_start(
                    out=tail_ktile,
                    in_=kt[:, slab_off[n_main][0] : slab_off[n_main][0] + tail_kv],
                )
            vtile_b = dma_v(k + 1, 0, slab_dims(k + 1)[1]) if has_b else None

            et = scores_exp(k, ktile_pair[:, 0:kvn])
            o_ps = new_o()
            pv(k, et, vtile, 0, n_t, o_ps, first=True)
            finalize(k, o_ps)
            if has_b:
                kvn_b, n_t_b, rem_b = slab_dims(k + 1)
                et_b = scores_exp(k + 1, ktile_pair[:, kvn : kvn + kvn_b])
                o_ps_b = new_o()
                pv(k + 1, et_b, vtile_b, 0, n_t_b, o_ps_b, first=True)
                finalize(k + 1, o_ps_b)

        # ---- tail: both slabs' scores first (their K arrived early), then
        # the PV chains chunk-by-chunk behind the V chunk arrivals ----
        if TAIL:
            ka, kb = n_main, n_main + 1
            dims_a, dims_b = slab_dims(ka), slab_dims(kb)
            kt_a = tail_ktile[:, 0 : dims_a[0]]
            kt_b = tail_ktile[:, dims_a[0] : dims_a[0] + dims_b[0]]

            def chunks_of(n_t):
                if n_t >= 4:
                    return [(0, n_t // 2), (n_t // 2, n_t)]
                return [(0, n_t)]

            ch_a = chunks_of(dims_a[1])
            ch_b = chunks_of(dims_b[1])
            v_a = [dma_v(ka, t0, t1) for t0, t1 in ch_a]
            v_b = [dma_v(kb, t0, t1) for t0, t1 in ch_b]
            et_a = scores_exp(ka, kt_a)
            et_b = scores_exp(kb, kt_b)
            o_a, o_b = new_o(), new_o()
            for i, (t0, t1) in enumerate(ch_a):
                pv(ka, et_a, v_a[i], t0, t1, o_a, first=(i == 0))
            finalize(ka, o_a)
            for i, (t0, t1) in enumerate(ch_b):
                pv(kb, et_b, v_b[i], t0, t1, o_b, first=(i == 0))
            finalize(kb, o_b)

        # out[r, slot, d]; two DMAs (on the otherwise-idle sync ring) so the
        # first half ships mid-kernel
        half = N_SLOT // 2
        nc.sync.dma_start(out=out[:, 0:half, :], in_=ostages[0])
        nc.sync.dma_start(out=out[:, half : N_SLOT, :], in_=ostages[1])


def build_nc(ext_tiles):
    sum_kv = sum(ext_tiles)
    sum_t = sum(-(-kvn // KV_TILE) for kvn in ext_tiles)
    nc = bacc.Bacc(
        "TRN2",
        target_bir_lowering=False,
        debug=False,
        num_devices=N_CORES,
    )
    ins = {
        "kt": nc.dram_tensor(
            "kt", [128, sum_kv], BF16, kind="ExternalInput"
        ).ap(),
        "vaug": nc.dram_tensor(
            "vaug", [128, sum_t, 129], BF16, kind="ExternalInput"
        ).ap(),
        "qt": nc.dram_tensor("qt", [D, N_SLOT * REP], F32, kind="ExternalInput").ap(),
        "ktn": nc.dram_tensor("ktn", [D, N_SLOT], F32, kind="ExternalInput").ap(),
        "vnew": nc.dram_tensor(
            "vnew", [1, N_SLOT * 129], F32, kind="ExternalInput"
        ).ap(),
    }
    outs = {
        "out": nc.dram_tensor(
            "out", [REP, N_SLOT, D], F32, kind="ExternalOutput"
        ).ap(),
    }
    with tile.TileContext(nc) as tc:
        _build_kernel_body(tc, ins, outs, ext_tiles)
    nc.compile()
    return nc


def plan_assignment(context_lens):
    """Slot k holds the k-th longest-context sequence (descending, so the
    final slab — the latency tail — is the smallest).  ext_kv[k] is that
    sequence's exact valid kv count (ctx-1); identical on every core.  The
    final 128-tile of each slab is partial: only ext_kv % 128 rows are
    loaded/computed."""
    context_lens = np.asarray(context_lens)
    slot_seq = list(np.argsort(-context_lens, kind="stable").astype(int))
    ext_kv = tuple(
        min(MAX_KV, max(1, int(context_lens[s]) - 1)) for s in slot_seq
    )
    return slot_seq, ext_kv


def make_in_maps(
    q, k, v, k_cache, v_cache, block_tables, context_lens, slot_mapping,
    slot_seq, ext_tiles,
):
    """Host-side sharding: gather each sequence's blocks from the paged cache
    once, lay K out transposed (d-major) and V kv-swizzled into (partition,
    tile) order, then split by kv-head across cores.  Pure data movement; the
    ones columns are constants.  slot_mapping is implied by context_lens for
    this problem's setup (slot == position ctx-1 in the gathered view)."""
    q = np.ascontiguousarray(np.asarray(q), dtype=np.float32)
    k = np.ascontiguousarray(np.asarray(k), dtype=np.float32)
    v = np.ascontiguousarray(np.asarray(v), dtype=np.float32)
    k_cache = np.asarray(k_cache)
    v_cache = np.asarray(v_cache)
    block_tables = np.asarray(block_tables)
    context_lens = np.asarray(context_lens)

    sum_kv = sum(ext_tiles)
    sum_t = sum(-(-kvn // KV_TILE) for kvn in ext_tiles)
    # staged in bf16: halves the HBM read volume vs f32 (the kernel's PE
    # operands are bf16 anyway, so the cast costs nothing extra on device)
    kt = [np.empty((128, sum_kv), ml_dtypes.bfloat16) for _ in range(N_CORES)]
    vaug = [
        np.empty((128, sum_t, 129), ml_dtypes.bfloat16) for _ in range(N_CORES)
    ]
    koff = 0
    voff = 0
    for slot, s in enumerate(slot_seq):
        kvn = ext_tiles[slot]
        n_t = -(-kvn // KV_TILE)
        # [256 blk, 16 pos, 8 g, 128 d] -> [kv, 8, 128]
        kg = k_cache[block_tables[s]].reshape(MAX_KV, KVH, D)[:kvn]
        vg = v_cache[block_tables[s]].reshape(MAX_KV, KVH, D)[: n_t * KV_TILE]
        kT = kg.transpose(1, 2, 0)                       # [8, 128 d, kvn]
        vsw = vg.reshape(n_t, KV_TILE, KVH, D).transpose(2, 1, 0, 3)  # [8,128p,t,d]
        for c in range(N_CORES):
            kt[c][:, koff : koff + kvn] = kT[c]
            vaug[c][:, voff : voff + n_t, :D] = vsw[c]
            vaug[c][:, voff : voff + n_t, D] = 1.0
        koff += kvn
        voff += n_t

    in_maps = []
    for c in range(N_CORES):
        # q^T for this core's 4 query heads of each slot's sequence
        qt = np.ascontiguousarray(
            q[slot_seq, c * REP : (c + 1) * REP, :]      # [16, 4, 128]
            .transpose(2, 0, 1)
            .reshape(D, N_SLOT * REP)
        )
        ktn = np.ascontiguousarray(k[slot_seq, c, :].T)   # [128, 16]
        vn = np.empty((N_SLOT, 129), np.float32)
        vn[:, :D] = v[slot_seq, c, :]
        vn[:, D] = 1.0
        in_maps.append(
            dict(
                kt=kt[c],
                vaug=vaug[c],
                qt=qt,
                ktn=ktn,
                vnew=np.ascontiguousarray(vn.reshape(1, N_SLOT * 129)),
            )
        )
    return in_maps


_NC_CACHE = {}


def get_nc(ext_tiles):
    if ext_tiles not in _NC_CACHE:
        _NC_CACHE[ext_tiles] = build_nc(ext_tiles)
    return _NC_CACHE[ext_tiles]


def kernel(q, k, v, k_cache, v_cache, block_tables, context_lens, slot_mapping):
    slot_seq, ext_tiles = plan_assignment(context_lens)
    in_maps = make_in_maps(
        q, k, v, k_cache, v_cache, block_tables, context_lens, slot_mapping,
        slot_seq, ext_tiles,
    )
    nc = get_nc(ext_tiles)
    res = None
    for attempt in range(3):
        try:
            res = run_bass_kernel_spmd(nc, in_maps, core_ids=list(range(N_CORES)))
            break
        except Exception:
            # transient NRT/device hiccups recover on a fresh dispatch
            if attempt == 2:
                raise
            time.sleep(5)
    return assemble_out(
        [np.asarray(res.results[i]["out"]) for i in range(N_CORES)], slot_seq
    )


def assemble_out(core_outs, slot_seq):
    """core c's out [r, slot, d] holds head (c*4+r) of sequence slot_seq[slot]."""
    out = np.empty((B, H, D), np.float32)
    for c, co in enumerate(core_outs):
        co = co.reshape(REP, N_SLOT, D)
        for slot, s in enumerate(slot_seq):
            out[s, c * REP : (c + 1) * REP, :] = co[:, slot, :]
    return out


if __name__ == "__main__":
    nc = build_nc(tuple([N_T] * N_SLOT))
    print("build OK")


# revision 23
# speedup vs baseline: 1.1200x; 1.1082x over previous
"""v0: original staged baseline + bf16 staging only (the 96.7us config)."""

import time

import ml_dtypes
import numpy as np

import concourse.bacc as bacc
import concourse.bass as bass
import concourse.tile as tile
from concourse import mybir
from concourse.bass_utils import run_bass_kernel_spmd

B, H, KVH, D = 16, 32, 8, 128
BLOCK_SIZE = 16
MAX_BLOCKS = 256
MAX_KV = MAX_BLOCKS * BLOCK_SIZE
SCALE = 1.0 / float(np.sqrt(D))
REP = H // KVH
N_CORES = 8
N_SLOT = B

F32 = mybir.dt.float32
BF16 = mybir.dt.bfloat16
I32 = mybir.dt.int32

KV_TILE = 128
N_T = MAX_KV // KV_TILE


def _build_kernel_body(tc, ins, outs, ext_tiles):
    nc = tc.nc
    kt = ins["kt"]
    vaug = ins["vaug"]
    qt = ins["qt"]
    ktn = ins["ktn"]
    vnew = ins["vnew"]
    out = outs["out"]

    with (
        tc.tile_pool(name="singles", bufs=1) as singles,
        tc.tile_pool(name="kpool", bufs=4) as kpool,
        tc.tile_pool(name="vpool", bufs=4) as vpool,
        tc.tile_pool(name="epool", bufs=2) as epool,
        tc.tile_pool(name="opool", bufs=4) as opool,
        tc.tile_pool(name="st_ps", bufs=2, space="PSUM") as st_ps,
        tc.tile_pool(name="o_ps", bufs=4, space="PSUM") as o_ps_pool,
        tc.tile_pool(name="snew_ps", bufs=1, space="PSUM") as snew_ps_pool,
    ):
        qtf = singles.tile([128, N_SLOT * REP], F32)
        nc.sync.dma_start(out=qtf, in_=qt)
        qtb = singles.tile([128, N_SLOT * REP], BF16)
        nc.vector.tensor_copy(out=qtb, in_=qtf)
        ktnf = singles.tile([128, N_SLOT], F32)
        nc.sync.dma_start(out=ktnf, in_=ktn)
        ktnb = singles.tile([128, N_SLOT], BF16)
        nc.vector.tensor_copy(out=ktnb, in_=ktnf)
        vnewf = singles.tile([1, N_SLOT * 129], F32)
        nc.sync.dma_start(out=vnewf, in_=vnew)
        vnewb = singles.tile([1, N_SLOT * 129], BF16)
        nc.vector.tensor_copy(out=vnewb, in_=vnewf)

        snew_ps = snew_ps_pool.tile([1, N_SLOT * REP], F32)
        for k in range(N_SLOT):
            nc.tensor.matmul(
                out=snew_ps[0:1, k * REP : (k + 1) * REP],
                lhsT=ktnb[:, k : k + 1],
                rhs=qtb[:, k * REP : (k + 1) * REP],
                start=(k == 0),
                stop=(k == N_SLOT - 1),
            )
        enew = singles.tile([1, N_SLOT * REP], BF16)
        nc.scalar.activation(
            out=enew, in_=snew_ps, func=mybir.ActivationFunctionType.Exp, scale=SCALE
        )

        OBASE = 64
        ost0_full = singles.tile([OBASE + REP, N_SLOT // 2, D], F32)
        ost1_full = singles.tile([OBASE + REP, N_SLOT // 2, D], F32)
        ostages = (
            ost0_full[OBASE : OBASE + REP],
            ost1_full[OBASE : OBASE + REP],
        )

        koff = 0
        voff = 0
        ktile_pair = None
        k_inner = 0
        for k in range(N_SLOT):
            kvn = ext_tiles[k]
            n_t = -(-kvn // KV_TILE)
            rem = kvn - (n_t - 1) * KV_TILE
            if k % 2 == 0:
                pair_kv = kvn + (ext_tiles[k + 1] if k + 1 < N_SLOT else 0)
                ktile_pair = kpool.tile([128, pair_kv], BF16, tag="ktile")
                nc.gpsimd.dma_start(
                    out=ktile_pair, in_=kt[:, koff : koff + pair_kv]
                )
                k_inner = 0
            ktile = ktile_pair[:, k_inner : k_inner + kvn]
            k_inner += kvn
            # one full-partition DMA per slab V: the partial last tile is
            # loaded in full (rows >= rem are zero padding, never read by
            # compute).  An exact [0:rem] partial DMA covers <8 partitions,
            # so it lands on 1-2 SDMA engines in sub-512B packets and was
            # measured drip-feeding for ~4us at the kernel tail.
            vtile = vpool.tile([128, n_t, 129], BF16, tag="vtile")
            nc.gpsimd.dma_start(
                out=vtile, in_=vaug[:, voff : voff + n_t, :]
            )

            st = st_ps.tile([128, n_t * REP], F32, tag="st")
            if n_t == 1:
                order = [0]
            else:
                order = [0, n_t - 1] + list(range(1, n_t - 1))
            stop_mm = None
            for i, t in enumerate(order):
                cols = KV_TILE if t < n_t - 1 else rem
                stop_mm = nc.tensor.matmul(
                    out=st[0:cols, t * REP : (t + 1) * REP],
                    lhsT=ktile[:, t * KV_TILE : t * KV_TILE + cols],
                    rhs=qtb[:, k * REP : (k + 1) * REP],
                    start=(i == 0),
                    stop=(i == len(order) - 1),
                )

            et = epool.tile([128, n_t * REP], BF16, tag="et")
            if n_t > 1:
                nc.scalar.activation(
                    out=et[:, 0 : (n_t - 1) * REP],
                    in_=st[:, 0 : (n_t - 1) * REP],
                    func=mybir.ActivationFunctionType.Exp,
                    scale=SCALE,
                )
            e_last = nc.scalar.activation(
                out=et[0:rem, (n_t - 1) * REP : n_t * REP],
                in_=st[0:rem, (n_t - 1) * REP : n_t * REP],
                func=mybir.ActivationFunctionType.Exp,
                scale=SCALE,
            )
            tile.add_dep_helper(
                e_last.ins, stop_mm.ins, reason="partial exp after group stop"
            )

            o_ps_full = o_ps_pool.tile([OBASE + REP, 129], F32, tag="o")
            o_ps = o_ps_full[OBASE : OBASE + REP]
            for t in range(n_t):
                kp = KV_TILE if t < n_t - 1 else rem
                nc.tensor.matmul(
                    out=o_ps,
                    lhsT=et[0:kp, t * REP : (t + 1) * REP],
                    rhs=vtile[0:kp, t, :],
                    start=(t == 0),
                    stop=False,
                )
            nc.tensor.matmul(
                out=o_ps,
                lhsT=enew[0:1, k * REP : (k + 1) * REP],
                rhs=vnewb[0:1, k * 129 : (k + 1) * 129],
                start=False,
                stop=True,
            )
            recip_full = opool.tile([OBASE + REP, 1], F32, tag="recip")
            recip = recip_full[OBASE : OBASE + REP]
            nc.vector.reciprocal(out=recip, in_=o_ps[:, 128:129])
            nc.vector.tensor_scalar_mul(
                out=ostages[k // (N_SLOT // 2)][:, k % (N_SLOT // 2), :],
                in0=o_ps[:, 0:128],
                scalar1=recip,
            )
            koff += kvn
            voff += n_t

        half = N_SLOT // 2
        nc.sync.dma_start(out=out[:, 0:half, :], in_=ostages[0])
        nc.sync.dma_start(out=out[:, half : N_SLOT, :], in_=ostages[1])


def build_nc(ext_tiles):
    sum_kv = sum(ext_tiles)
    sum_t = sum(-(-kvn // KV_TILE) for kvn in ext_tiles)
    nc = bacc.Bacc(
        "TRN2",
        target_bir_lowering=False,
        debug=False,
        num_devices=N_CORES,
    )
    ins = {
        "kt": nc.dram_tensor(
            "kt", [128, sum_kv], BF16, kind="ExternalInput"
        ).ap(),
        "vaug": nc.dram_tensor(
            "vaug", [128, sum_t, 129], BF16, kind="ExternalInput"
        ).ap(),
        "qt": nc.dram_tensor("qt", [D, N_SLOT * REP], F32, kind="ExternalInput").ap(),
        "ktn": nc.dram_tensor("ktn", [D, N_SLOT], F32, kind="ExternalInput").ap(),
        "vnew": nc.dram_tensor(
            "vnew", [1, N_SLOT * 129], F32, kind="ExternalInput"
        ).ap(),
    }
    outs = {
        "out": nc.dram_tensor(
            "out", [REP, N_SLOT, D], F32, kind="ExternalOutput"
        ).ap(),
    }
    with tile.TileContext(nc) as tc:
        _build_kernel_body(tc, ins, outs, ext_tiles)
    nc.compile()
    return nc


def plan_assignment(context_lens):
    context_lens = np.asarray(context_lens)
    slot_seq = list(np.argsort(-context_lens, kind="stable").astype(int))
    ext_kv = tuple(
        min(MAX_KV, max(1, int(context_lens[s]) - 1)) for s in slot_seq
    )
    return slot_seq, ext_kv


def make_in_maps(
    q, k, v, k_cache, v_cache, block_tables, context_lens, slot_mapping,
    slot_seq, ext_tiles,
):
    q = np.ascontiguousarray(np.asarray(q), dtype=np.float32)
    k = np.ascontiguousarray(np.asarray(k), dtype=np.float32)
    v = np.ascontiguousarray(np.asarray(v), dtype=np.float32)
    k_cache = np.asarray(k_cache)
    v_cache = np.asarray(v_cache)
    block_tables = np.asarray(block_tables)
    context_lens = np.asarray(context_lens)

    sum_kv = sum(ext_tiles)
    sum_t = sum(-(-kvn // KV_TILE) for kvn in ext_tiles)
    kt = [np.empty((128, sum_kv), ml_dtypes.bfloat16) for _ in range(N_CORES)]
    # zeros (not empty): the kernel DMA-loads the padding rows of each
    # slab's partial last V tile, so they must hold benign values
    vaug = [
        np.zeros((128, sum_t, 129), ml_dtypes.bfloat16) for _ in range(N_CORES)
    ]
    koff = 0
    voff = 0
    for slot, s in enumerate(slot_seq):
        kvn = ext_tiles[slot]
        n_t = -(-kvn // KV_TILE)
        kg = k_cache[block_tables[s]].reshape(MAX_KV, KVH, D)[:kvn]
        vg = v_cache[block_tables[s]].reshape(MAX_KV, KVH, D)[: n_t * KV_TILE]
        kT = kg.transpose(1, 2, 0)
        vsw = vg.reshape(n_t, KV_TILE, KVH, D).transpose(2, 1, 0, 3)
        for c in range(N_CORES):
            kt[c][:, koff : koff + kvn] = kT[c]
            vaug[c][:, voff : voff + n_t, :D] = vsw[c]
            vaug[c][:, voff : voff + n_t, D] = 1.0
        koff += kvn
        voff += n_t

    in_maps = []
    for c in range(N_CORES):
        qt = np.ascontiguousarray(
            q[slot_seq, c * REP : (c + 1) * REP, :]
            .transpose(2, 0, 1)
            .reshape(D, N_SLOT * REP)
        )
        ktn = np.ascontiguousarray(k[slot_seq, c, :].T)
        vn = np.empty((N_SLOT, 129), np.float32)
        vn[:, :D] = v[slot_seq, c, :]
        vn[:, D] = 1.0
        in_maps.append(
            dict(
                kt=kt[c],
                vaug=vaug[c],
                qt=qt,
                ktn=ktn,
                vnew=np.ascontiguousarray(vn.reshape(1, N_SLOT * 129)),
            )
        )
    return in_maps


_NC_CACHE = {}


def get_nc(ext_tiles):
    if ext_tiles not in _NC_CACHE:
        _NC_CACHE[ext_tiles] = build_nc(ext_tiles)
    return _NC_CACHE[ext_tiles]


def kernel(q, k, v, k_cache, v_cache, block_tables, context_lens, slot_mapping):
    slot_seq, ext_tiles = plan_assignment(context_lens)
    in_maps = make_in_maps(
        q, k, v, k_cache, v_cache, block_tables, context_lens, slot_mapping,
        slot_seq, ext_tiles,
    )
    nc = get_nc(ext_tiles)
    res = None
    for attempt in range(3):
        try:
            res = run_bass_kernel_spmd(nc, in_maps, core_ids=list(range(N_CORES)))
            break
        except Exception:
            if attempt == 2:
                raise
            time.sleep(5)
    return assemble_out(
        [np.asarray(res.results[i]["out"]) for i in range(N_CORES)], slot_seq
    )


def assemble_out(core_outs, slot_seq):
    out = np.empty((B, H, D), np.float32)
    for c, co in enumerate(core_outs):
        co = co.reshape(REP, N_SLOT, D)
        for slot, s in enumerate(slot_seq):
            out[s, c * REP : (c + 1) * REP, :] = co[:, slot, :]
    return out


if __name__ == "__main__":
    nc = build_nc(tuple([N_T] * N_SLOT))
    print("build OK")


# revision 24
# speedup vs baseline: 1.1247x; 1.0041x over previous
"""Paged-attention decode (GQA, vLLM-style) on 8 TRN2 NeuronCores.

Sharding: kv-head-parallel - core c owns kv-head c (and its 4 query heads)
for ALL 16 sequences; no collectives.  Each core processes 16 slabs, one per
(sequence, head) unit, in descending context-length order; a slab's kv
extent is exactly ctx-1 valid rows, so invalid kv is never loaded and no
masking is needed.  The graph is compiled per extent tuple (cached);
extents are shared across cores.  Host side does only data movement
(gather per block_tables, layout transforms, f32->bf16 staging).

Performance notes (measured on HW, 8 cores concurrent):
- K/V staged in DRAM as bf16: halves the HBM read volume (~26 MB/core);
  the stream runs at the ~358 GB/s per-core HBM roofline (~74 us).
- ONE SWDGE queue in sequential DRAM address order.  Splitting K and V onto
  concurrent queues measured 20% slower (296 vs 368 GB/s): two interleaved
  HBM address streams defeat row locality.  Same for HWDGE head prefetch.
- V tiles are loaded full-partition, one DMA per slab: an exact [0:rem]
  partial-tile DMA covers <8 partitions, lands on 1-2 SDMA engines in
  sub-512B packets, and was measured drip-feeding ~4 us at the kernel tail
  (the padding rows are zeros host-side and never read by compute).
- The PE tail chain runs at ~107 ns/tile (instruction-overhead/HAM-cold
  bound); reordering or splitting the tail slabs does not beat the simple
  descending-size schedule.

Device algorithm per slab (one sequence, one kv-head, REP=4 query heads):
  - scores^T tiles  S^T[kv,r] = sum_d K[kv,d] Q[r,d]  via PE matmuls with
    the K tile as the (transposed-layout) stationary operand, PSUM-accum.
  - E = exp(S * scale) on ScalarE straight out of PSUM (no max-subtraction:
    |scores| <= ~6 so bf16 exp is safe; 3e-3 rel err end to end).
  - out = (E^T @ [V | 1]) -> [4, 129]; column 128 accumulates the softmax
    denominator for free (ones column appended to V on host).
  - new token at position ctx-1 handled separately: one small matmul
    against k_new, exp, then a K=1 matmul accumulates e_new * [v_new | 1]
    into the same PSUM group.  Finally out[:, :128] / out[:, 128] -> DRAM.
"""

import time

import ml_dtypes
import numpy as np

import concourse.bacc as bacc
import concourse.bass as bass
import concourse.tile as tile
from concourse import mybir
from concourse.bass_utils import run_bass_kernel_spmd

B, H, KVH, D = 16, 32, 8, 128
BLOCK_SIZE = 16
MAX_BLOCKS = 256
MAX_KV = MAX_BLOCKS * BLOCK_SIZE
SCALE = 1.0 / float(np.sqrt(D))
REP = H // KVH
N_CORES = 8
N_SLOT = B

F32 = mybir.dt.float32
BF16 = mybir.dt.bfloat16
I32 = mybir.dt.int32

KV_TILE = 128
N_T = MAX_KV // KV_TILE


def _build_kernel_body(tc, ins, outs, ext_tiles):
    nc = tc.nc
    kt = ins["kt"]
    vaug = ins["vaug"]
    qt = ins["qt"]
    ktn = ins["ktn"]
    vnew = ins["vnew"]
    out = outs["out"]

    with (
        tc.tile_pool(name="singles", bufs=1) as singles,
        tc.tile_pool(name="kpool", bufs=4) as kpool,
        tc.tile_pool(name="vpool", bufs=4) as vpool,
        tc.tile_pool(name="epool", bufs=2) as epool,
        tc.tile_pool(name="opool", bufs=4) as opool,
        tc.tile_pool(name="st_ps", bufs=2, space="PSUM") as st_ps,
        tc.tile_pool(name="o_ps", bufs=4, space="PSUM") as o_ps_pool,
        tc.tile_pool(name="snew_ps", bufs=1, space="PSUM") as snew_ps_pool,
    ):
        qtf = singles.tile([128, N_SLOT * REP], F32)
        nc.sync.dma_start(out=qtf, in_=qt)
        qtb = singles.tile([128, N_SLOT * REP], BF16)
        nc.vector.tensor_copy(out=qtb, in_=qtf)
        ktnf = singles.tile([128, N_SLOT], F32)
        nc.sync.dma_start(out=ktnf, in_=ktn)
        ktnb = singles.tile([128, N_SLOT], BF16)
        nc.vector.tensor_copy(out=ktnb, in_=ktnf)
        vnewf = singles.tile([1, N_SLOT * 129], F32)
        nc.sync.dma_start(out=vnewf, in_=vnew)
        vnewb = singles.tile([1, N_SLOT * 129], BF16)
        nc.vector.tensor_copy(out=vnewb, in_=vnewf)

        snew_ps = snew_ps_pool.tile([1, N_SLOT * REP], F32)
        for k in range(N_SLOT):
            nc.tensor.matmul(
                out=snew_ps[0:1, k * REP : (k + 1) * REP],
                lhsT=ktnb[:, k : k + 1],
                rhs=qtb[:, k * REP : (k + 1) * REP],
                start=(k == 0),
                stop=(k == N_SLOT - 1),
            )
        enew = singles.tile([1, N_SLOT * REP], BF16)
        nc.scalar.activation(
            out=enew, in_=snew_ps, func=mybir.ActivationFunctionType.Exp, scale=SCALE
        )

        OBASE = 64
        ost0_full = singles.tile([OBASE + REP, N_SLOT // 2, D], F32)
        ost1_full = singles.tile([OBASE + REP, N_SLOT // 2, D], F32)
        ostages = (
            ost0_full[OBASE : OBASE + REP],
            ost1_full[OBASE : OBASE + REP],
        )

        koff = 0
        voff = 0
        ktile_pair = None
        k_inner = 0
        for k in range(N_SLOT):
            kvn = ext_tiles[k]
            n_t = -(-kvn // KV_TILE)
            rem = kvn - (n_t - 1) * KV_TILE
            if k % 2 == 0:
                pair_kv = kvn + (ext_tiles[k + 1] if k + 1 < N_SLOT else 0)
                ktile_pair = kpool.tile([128, pair_kv], BF16, tag="ktile")
                nc.gpsimd.dma_start(
                    out=ktile_pair, in_=kt[:, koff : koff + pair_kv]
                )
                k_inner = 0
            ktile = ktile_pair[:, k_inner : k_inner + kvn]
            k_inner += kvn
            # one full-partition DMA per slab V: the partial last tile is
            # loaded in full (rows >= rem are zero padding, never read by
            # compute).  An exact [0:rem] partial DMA covers <8 partitions,
            # so it lands on 1-2 SDMA engines in sub-512B packets and was
            # measured drip-feeding for ~4us at the kernel tail.
            vtile = vpool.tile([128, n_t, 129], BF16, tag="vtile")
            nc.gpsimd.dma_start(
                out=vtile, in_=vaug[:, voff : voff + n_t, :]
            )

            st = st_ps.tile([128, n_t * REP], F32, tag="st")
            if n_t == 1:
                order = [0]
            else:
                order = [0, n_t - 1] + list(range(1, n_t - 1))
            stop_mm = None
            for i, t in enumerate(order):
                cols = KV_TILE if t < n_t - 1 else rem
                stop_mm = nc.tensor.matmul(
                    out=st[0:cols, t * REP : (t + 1) * REP],
                    lhsT=ktile[:, t * KV_TILE : t * KV_TILE + cols],
                    rhs=qtb[:, k * REP : (k + 1) * REP],
                    start=(i == 0),
                    stop=(i == len(order) - 1),
                )

            et = epool.tile([128, n_t * REP], BF16, tag="et")
            if n_t > 1:
                nc.scalar.activation(
                    out=et[:, 0 : (n_t - 1) * REP],
                    in_=st[:, 0 : (n_t - 1) * REP],
                    func=mybir.ActivationFunctionType.Exp,
                    scale=SCALE,
                )
            e_last = nc.scalar.activation(
                out=et[0:rem, (n_t - 1) * REP : n_t * REP],
                in_=st[0:rem, (n_t - 1) * REP : n_t * REP],
                func=mybir.ActivationFunctionType.Exp,
                scale=SCALE,
            )
            tile.add_dep_helper(
                e_last.ins, stop_mm.ins, reason="partial exp after group stop"
            )

            o_ps_full = o_ps_pool.tile([OBASE + REP, 129], F32, tag="o")
            o_ps = o_ps_full[OBASE : OBASE + REP]
            for t in range(n_t):
                kp = KV_TILE if t < n_t - 1 else rem
                nc.tensor.matmul(
                    out=o_ps,
                    lhsT=et[0:kp, t * REP : (t + 1) * REP],
                    rhs=vtile[0:kp, t, :],
                    start=(t == 0),
                    stop=False,
                )
            nc.tensor.matmul(
                out=o_ps,
                lhsT=enew[0:1, k * REP : (k + 1) * REP],
                rhs=vnewb[0:1, k * 129 : (k + 1) * 129],
                start=False,
                stop=True,
            )
            recip_full = opool.tile([OBASE + REP, 1], F32, tag="recip")
            recip = recip_full[OBASE : OBASE + REP]
            nc.vector.reciprocal(out=recip, in_=o_ps[:, 128:129])
            nc.vector.tensor_scalar_mul(
                out=ostages[k // (N_SLOT // 2)][:, k % (N_SLOT // 2), :],
                in0=o_ps[:, 0:128],
                scalar1=recip,
            )
            koff += kvn
            voff += n_t

        half = N_SLOT // 2
        nc.sync.dma_start(out=out[:, 0:half, :], in_=ostages[0])
        nc.sync.dma_start(out=out[:, half : N_SLOT, :], in_=ostages[1])


def build_nc(ext_tiles):
    sum_kv = sum(ext_tiles)
    sum_t = sum(-(-kvn // KV_TILE) for kvn in ext_tiles)
    nc = bacc.Bacc(
        "TRN2",
        target_bir_lowering=False,
        debug=False,
        num_devices=N_CORES,
    )
    ins = {
        "kt": nc.dram_tensor(
            "kt", [128, sum_kv], BF16, kind="ExternalInput"
        ).ap(),
        "vaug": nc.dram_tensor(
            "vaug", [128, sum_t, 129], BF16, kind="ExternalInput"
        ).ap(),
        "qt": nc.dram_tensor("qt", [D, N_SLOT * REP], F32, kind="ExternalInput").ap(),
        "ktn": nc.dram_tensor("ktn", [D, N_SLOT], F32, kind="ExternalInput").ap(),
        "vnew": nc.dram_tensor(
            "vnew", [1, N_SLOT * 129], F32, kind="ExternalInput"
        ).ap(),
    }
    outs = {
        "out": nc.dram_tensor(
            "out", [REP, N_SLOT, D], F32, kind="ExternalOutput"
        ).ap(),
    }
    with tile.TileContext(nc) as tc:
        _build_kernel_body(tc, ins, outs, ext_tiles)
    nc.compile()
    return nc


def plan_assignment(context_lens):
    context_lens = np.asarray(context_lens)
    slot_seq = list(np.argsort(-context_lens, kind="stable").astype(int))
    ext_kv = tuple(
        min(MAX_KV, max(1, int(context_lens[s]) - 1)) for s in slot_seq
    )
    return slot_seq, ext_kv


def make_in_maps(
    q, k, v, k_cache, v_cache, block_tables, context_lens, slot_mapping,
    slot_seq, ext_tiles,
):
    q = np.ascontiguousarray(np.asarray(q), dtype=np.float32)
    k = np.ascontiguousarray(np.asarray(k), dtype=np.float32)
    v = np.ascontiguousarray(np.asarray(v), dtype=np.float32)
    k_cache = np.asarray(k_cache)
    v_cache = np.asarray(v_cache)
    block_tables = np.asarray(block_tables)
    context_lens = np.asarray(context_lens)

    sum_kv = sum(ext_tiles)
    sum_t = sum(-(-kvn // KV_TILE) for kvn in ext_tiles)
    kt = [np.empty((128, sum_kv), ml_dtypes.bfloat16) for _ in range(N_CORES)]
    # zeros (not empty): the kernel DMA-loads the padding rows of each
    # slab's partial last V tile, so they must hold benign values
    vaug = [
        np.zeros((128, sum_t, 129), ml_dtypes.bfloat16) for _ in range(N_CORES)
    ]
    koff = 0
    voff = 0
    for slot, s in enumerate(slot_seq):
        kvn = ext_tiles[slot]
        n_t = -(-kvn // KV_TILE)
        kg = k_cache[block_tables[s]].reshape(MAX_KV, KVH, D)[:kvn]
        vg = v_cache[block_tables[s]].reshape(MAX_KV, KVH, D)[: n_t * KV_TILE]
        kT = kg.transpose(1, 2, 0)
        vsw = vg.reshape(n_t, KV_TILE, KVH, D).transpose(2, 1, 0, 3)
        for c in range(N_CORES):
            kt[c][:, koff : koff + kvn] = kT[c]
            vaug[c][:, voff : voff + n_t, :D] = vsw[c]
            vaug[c][:, voff : voff + n_t, D] = 1.0
        koff += kvn
        voff += n_t

    in_maps = []
    for c in range(N_CORES):
        qt = np.ascontiguousarray(
            q[slot_seq, c * REP : (c + 1) * REP, :]
            .transpose(2, 0, 1)
            .reshape(D, N_SLOT * REP)
        )
        ktn = np.ascontiguousarray(k[slot_seq, c, :].T)
        vn = np.empty((N_SLOT, 129), np.float32)
        vn[:, :D] = v[slot_seq, c, :]
        vn[:, D] = 1.0
        in_maps.append(
            dict(
                kt=kt[c],
                vaug=vaug[c],
                qt=qt,
                ktn=ktn,
                vnew=np.ascontiguousarray(vn.reshape(1, N_SLOT * 129)),
            )
        )
    return in_maps


_NC_CACHE = {}


def get_nc(ext_tiles):
    if ext_tiles not in _NC_CACHE:
        _NC_CACHE[ext_tiles] = build_nc(ext_tiles)
    return _NC_CACHE[ext_tiles]


def kernel(q, k, v, k_cache, v_cache, block_tables, context_lens, slot_mapping):
    slot_seq, ext_tiles = plan_assignment(context_lens)
    in_maps = make_in_maps(
        q, k, v, k_cache, v_cache, block_tables, context_lens, slot_mapping,
        slot_seq, ext_tiles,
    )
    nc = get_nc(ext_tiles)
    res = None
    for attempt in range(3):
        try:
            res = run_bass_kernel_spmd(nc, in_maps, core_ids=list(range(N_CORES)))
            break
        except Exception:
            if attempt == 2:
                raise
            time.sleep(5)
    return assemble_out(
        [np.asarray(res.results[i]["out"]) for i in range(N_CORES)], slot_seq
    )


def assemble_out(core_outs, slot_seq):
    out = np.empty((B, H, D), np.float32)
    for c, co in enumerate(core_outs):
        co = co.reshape(REP, N_SLOT, D)
        for slot, s in enumerate(slot_seq):
            out[s, c * REP : (c + 1) * REP, :] = co[:, slot, :]
    return out


if __name__ == "__main__":
    nc = build_nc(tuple([N_T] * N_SLOT))
    print("build OK")
